# revision 1
# baseline (speedup 1.0000x reference)
"""Causal self-attention (B=4, T=2048, C=1024, H=16) on 8 Trainium2 NeuronCores.

Sharding: core = (batch b = core//2, head-group g = core%2, 8 heads each).
Per core:
  - QKV projection for its 512 q/k/v channels: fp32r matmuls (full PE rate).
    The 1/sqrt(D) scale and biases fold into the PSUM evacuation (DVE
    tensor_scalar), reserving ScalarE for exp.
  - Attention per head pair: S^T = K^T.T @ Q^T row-tiled (the two heads use
    disjoint 64-row groups of the PE array and run concurrently), one exp per
    k-tile over a merged 2-bank PSUM tile, causal tril mask multiplied on DVE
    for diagonal tiles only (above-diagonal tiles skipped; diagonal tiles are
    restricted to their valid q-range), PV matmul accumulates with a
    ones*pad column appended to V' so row 64 of the accumulator is the
    softmax denominator (padding mask folded into V' at zero per-tile cost).
    The k-loop is software pipelined: S/exp run one step ahead of PV.
  - Normalize: DVE copy (partition crossbar 64->0), reciprocal_approx_fast,
    gpsimd partition-broadcast, DVE multiply into y^T.
  - Output projection rows slice -> partial [T, C] output.
Host: transposes x per batch, slices Wqkv/Wproj by head group, sums the two
partials per batch and adds bproj.
"""

import os
import sys

for _p in ("/opt/trn_rl_repo",):
    if _p not in sys.path:
        sys.path.append(_p)

import numpy as np

B, T, C = 4, 2048, 1024
H, D = 16, 64
HPC = 8          # heads per core
GC = HPC * D     # 512 channels per core
N_CORES = 8
P = 128
NT = T // 512    # 4  q-tiles / n-slices of 512
MT = GC // 128   # 4  m-tiles (head pairs)
CT = C // 128    # 8  contraction tiles
TT = T // 128    # 16 t-tiles of 128

_cached = {}


def _build():
    import concourse.tile as tile
    from concourse import bacc, mybir
    import concourse.bass as bass

    f32 = mybir.dt.float32
    f32r = mybir.dt.float32r
    bf16 = mybir.dt.bfloat16
    AF = mybir.ActivationFunctionType
    ADD = mybir.AluOpType.add
    MUL = mybir.AluOpType.mult

    nc = bacc.Bacc("TRN2", target_bir_lowering=False, debug=False)

    xT_d = nc.dram_tensor("xT", [C, T], f32, kind="ExternalInput")
    wq_d = nc.dram_tensor("wq", [C, GC], f32, kind="ExternalInput")
    wk_d = nc.dram_tensor("wk", [C, GC], f32, kind="ExternalInput")
    wv_d = nc.dram_tensor("wv", [C, GC], f32, kind="ExternalInput")
    bq_d = nc.dram_tensor("bq", [GC], f32, kind="ExternalInput")
    bk_d = nc.dram_tensor("bk", [GC], f32, kind="ExternalInput")
    bv_d = nc.dram_tensor("bv", [GC], f32, kind="ExternalInput")
    wp_d = nc.dram_tensor("wp", [GC, C], f32, kind="ExternalInput")
    pad_d = nc.dram_tensor("pad", [T], f32, kind="ExternalInput")
    mask_d = nc.dram_tensor("mask", [P, 512], f32, kind="ExternalInput")
    out_d = nc.dram_tensor("out", [T, C], f32, kind="ExternalOutput")

    with tile.TileContext(nc) as tc:
        with tc.tile_pool(name="persist", bufs=1) as persist, \
             tc.tile_pool(name="allps", bufs=2, space="PSUM") as allps:
            QT = persist.tile([P, MT, T], f32r, tag="QT")
            KT = persist.tile([P, MT, T], f32r, tag="KT")
            Vp = persist.tile([P, TT, HPC, D + 1], f32r, tag="Vp")
            pad_s = persist.tile([P, TT], f32, tag="pad")
            bq_s = persist.tile([P, MT], f32, tag="bq")
            bk_s = persist.tile([P, MT], f32, tag="bk")
            bv_s = persist.tile([P, GC], f32, tag="bv")
            tril_s = persist.tile([P, 512], f32r, tag="tril")

            nc.sync.dma_start(pad_s[:], pad_d.rearrange("(tt p) -> p tt", p=P))
            nc.sync.dma_start(bq_s[:], bq_d.rearrange("(m p) -> p m", p=P))
            nc.sync.dma_start(bk_s[:], bk_d.rearrange("(m p) -> p m", p=P))
            bv_ap = bass.AP(tensor=bv_d[:].tensor, offset=0, ap=[[0, P], [1, GC]])
            nc.sync.dma_start(bv_s[:], bv_ap)
            nc.sync.dma_start(tril_s[:], mask_d[:].bitcast(f32r))
            # Vp pad column: Vp[:, tt, h, 64] = pad[tt*128 + p] for all h
            for tt in range(TT):
                nc.vector.memset(Vp[:, tt, :, D:D + 1].bitcast(f32), 1.0)
                nc.vector.tensor_scalar(
                    out=Vp[:, tt, :, D:D + 1], in0=Vp[:, tt, :, D:D + 1],
                    scalar1=pad_s[:, tt:tt + 1], scalar2=None, op0=MUL)

            xTr = xT_d.rearrange("(c p) t -> p c t", p=P).bitcast(f32r)

            # ---------------- phase 1: V + Q^T + K^T projections ----------
            with tc.tile_pool(name="wpool", bufs=1) as wpool, \
                 tc.tile_pool(name="xpool", bufs=2) as xpool, \
                 tc.tile_pool(name="tpool", bufs=2) as tpool:
                # first x slice before the weights so PE work starts early;
                # chunked DMAs so the first matmuls' inputs land quickly
                xtn0 = xpool.tile([P, CT, 512], f32r, tag="xtn")
                for c2 in range(0, CT, 2):
                    nc.sync.dma_start(
                        xtn0[:, c2:c2 + 2, :], xTr[:, c2:c2 + 2, 0:512])
                wv_s = wpool.tile([P, CT, GC], f32r, tag="wv")
                wk_s = wpool.tile([P, CT, GC], f32r, tag="wk")
                wq_s = wpool.tile([P, CT, GC], f32r, tag="wq")
                wvr = wv_d.rearrange("(c p) n -> p c n", p=P).bitcast(f32r)
                wkr = wk_d.rearrange("(c p) n -> p c n", p=P).bitcast(f32r)
                wqr = wq_d.rearrange("(c p) n -> p c n", p=P).bitcast(f32r)
                for c2 in range(0, CT, 2):
                    nc.sync.dma_start(wv_s[:, c2:c2 + 2, :], wvr[:, c2:c2 + 2, :])
                for c2 in range(0, CT, 2):
                    nc.sync.dma_start(wk_s[:, c2:c2 + 2, :], wkr[:, c2:c2 + 2, :])
                for c2 in range(0, CT, 2):
                    nc.sync.dma_start(wq_s[:, c2:c2 + 2, :], wqr[:, c2:c2 + 2, :])

                for nt in range(NT):
                    if nt == 0:
                        xt_n = xtn0
                    else:
                        xt_n = xpool.tile([P, CT, 512], f32r, tag="xtn")
                        nc.sync.dma_start(xt_n[:], xTr[:, :, nt * 512:(nt + 1) * 512])
                    # V for 4 t-subtiles of this n-slice
                    for ts in range(4):
                        tt = nt * 4 + ts
                        ps = allps.tile([P, GC], f32, tag="SS")
                        for c in range(CT):
                            nc.tensor.matmul(
                                ps[:], xt_n[:, c, ts * P:(ts + 1) * P], wv_s[:, c, :],
                                start=(c == 0), stop=(c == CT - 1))
                        tmp = tpool.tile([P, GC], f32, tag="vtmp")
                        nc.vector.tensor_add(tmp[:], ps[:], bv_s[:])
                        nc.vector.tensor_scalar(
                            out=Vp[:, tt, :, 0:D],
                            in0=tmp[:].rearrange("p (h d) -> p h d", h=HPC),
                            scalar1=pad_s[:, tt:tt + 1], scalar2=None, op0=MUL)
                    # Q^T and K^T m-tiles for this n-slice
                    for W, bias, OUT, qscale in (
                            (wq_s, bq_s, QT, True), (wk_s, bk_s, KT, False)):
                        for m in range(MT):
                            ps = allps.tile([P, 512], f32, tag="SS")
                            for c in range(CT):
                                nc.tensor.matmul(
                                    ps[:], W[:, c, m * P:(m + 1) * P], xt_n[:, c, :],
                                    start=(c == 0), stop=(c == CT - 1))
                            if qscale:
                                nc.vector.tensor_scalar(
                                    out=OUT[:, m, nt * 512:(nt + 1) * 512], in0=ps[:],
                                    scalar1=bias[:, m:m + 1], scalar2=0.125,
                                    op0=ADD, op1=MUL)
                            else:
                                nc.vector.tensor_scalar(
                                    out=OUT[:, m, nt * 512:(nt + 1) * 512], in0=ps[:],
                                    scalar1=bias[:, m:m + 1], scalar2=None, op0=ADD)

            # ---------------- phase 2: attention + projection -------------
            with tc.tile_pool(name="apool", bufs=1) as apool, \
                 tc.tile_pool(name="ypool", bufs=2) as ypool, \
                 tc.tile_pool(name="ppool", bufs=6) as ppool, \
                 tc.tile_pool(name="bpool", bufs=2) as bpool, \
                 tc.tile_pool(name="prpool", bufs=3) as prpool:
                wp_s = apool.tile([P, MT, C], f32r, tag="wp")
                nc.sync.dma_start(wp_s[:], wp_d.rearrange("(m p) n -> p m n", p=P).bitcast(f32r))

                def proj_granule(qt_, yT_, ts, nh, half, ps):
                    # half a projection tile (2 of 4 contraction matmuls)
                    tt = qt_ * 4 + ts
                    for cj in (0, 1) if half == 0 else (2, 3):
                        nc.tensor.matmul(
                            ps[:], yT_[:, cj, ts * P:(ts + 1) * P],
                            wp_s[:, cj, nh * 512:(nh + 1) * 512],
                            start=(cj == 0), stop=(cj == MT - 1))
                    if half == 1:
                        ot = prpool.tile([P, 512], f32, tag="ot")
                        nc.vector.tensor_copy(ot[:], ps[:])
                        nc.sync.dma_start(
                            out_d[tt * P:(tt + 1) * P, nh * 512:(nh + 1) * 512], ot[:])

                def proj_piece(qt_, yT_, ts):
                    for nh in range(2):
                        ps = allps.tile([P, 512], f32, tag="OO")
                        proj_granule(qt_, yT_, ts, nh, 0, ps)
                        proj_granule(qt_, yT_, ts, nh, 1, ps)

                yTq_prev = None
                for qt in range(NT):
                    yTq = ypool.tile([P, MT, 512], f32r, tag="yTq")
                    nk = 4 * (qt + 1)
                    # flattened cross-pair pipeline: S/exp of pair j+1 issue
                    # before pair j's PV tail drains, so ScalarE never idles
                    # at pair boundaries
                    OO_map = {}
                    pend = []
                    LAG = 3

                    def normalize_and_aux(j_, qt=qt, yTq=yTq, yTq_prev=yTq_prev):
                        OO_ = OO_map[j_]
                        # l rows -> partition 0, recip, bcast, mul
                        lraw = bpool.tile([1, 2, 512], f32, tag="lraw")
                        lrec = bpool.tile([1, 2, 512], f32, tag="lrec")
                        nc.vector.tensor_copy(lraw[0:1, :, :], OO_[D:D + 1, :, :])
                        nc.vector.reciprocal_approx_fast(lrec[0:1, :, :], lraw[0:1, :, :])
                        bc = bpool.tile([P, 2, 512], f32, tag="bc")
                        nc.gpsimd.partition_broadcast(bc[:], lrec[0:1, :, :], channels=P)
                        nc.vector.tensor_mul(yTq[0:D, j_, :], OO_[0:D, 0, :], bc[0:D, 0, :])
                        nc.vector.tensor_mul(yTq[D:P, j_, :], OO_[0:D, 1, :], bc[D:P, 1, :])
                        if yTq_prev is not None:
                            proj_piece(qt - 1, yTq_prev, j_)

                    def emit_pv(entry, nk=nk):
                        j_, k_, z_, PP_ = entry
                        OO_ = OO_map[j_]
                        last = (k_ == nk - 1)
                        for e in range(2):
                            nc.tensor.matmul(
                                OO_[:, e, z_:512], Vp[:, k_, 2 * j_ + e, :],
                                PP_[:, e, z_:512],
                                start=(k_ == 0), stop=last)
                        if last:
                            normalize_and_aux(j_)

                    for j in range(MT):
                        OO_map[j] = allps.tile(
                            [D + 1, 2, 512], f32, tag="OO", name="OO")
                        for kt in range(nk):
                            off = kt - 4 * qt
                            q0 = max(off, 0) * P
                            SS = allps.tile([P, 2, 512], f32, tag="SS")
                            nc.tensor.matmul(
                                SS[:, 0, q0:512], KT[0:D, j, kt * P:(kt + 1) * P],
                                QT[0:D, j, qt * 512 + q0:(qt + 1) * 512],
                                start=True, stop=True)
                            nc.tensor.matmul(
                                SS[:, 1, q0:512], KT[D:P, j, kt * P:(kt + 1) * P],
                                QT[D:P, j, qt * 512 + q0:(qt + 1) * 512],
                                start=True, stop=True)
                            PP = ppool.tile([P, 2, 512], f32r, tag="PP")
                            nc.scalar.activation(
                                PP[:, :, q0:512], SS[:, :, q0:512], AF.Exp)
                            if off >= 0:
                                # causal prefix of the tril mask, bcast over heads
                                tm = tril_s[:, 0:512 - q0]
                                mask_b = bass.AP(
                                    tensor=tm.tensor, offset=tm.offset,
                                    ap=[list(tm.ap[0]), [0, 2], list(tm.ap[1])])
                                nc.vector.tensor_mul(
                                    PP[:, :, q0:512], PP[:, :, q0:512], mask_b)
                            pend.append((j, kt, q0, PP))
                            if len(pend) > LAG:
                                emit_pv(pend.pop(0))
                    while pend:
                        emit_pv(pend.pop(0))
                    yTq_prev = yTq
                # final q block's projection
                for ts in range(4):
                    proj_piece(NT - 1, yTq_prev, ts)

    nc.compile()
    return nc


def _get_nc():
    if "nc" not in _cached:
        _cached["nc"] = _build()
    return _cached["nc"]


def kernel(x, padding_mask, Wqkv, bqkv, Wproj, bproj):
    from concourse.bass_utils import run_bass_kernel_spmd

    x = np.asarray(x, dtype=np.float32)
    padding_mask = np.asarray(padding_mask)
    Wqkv = np.asarray(Wqkv, dtype=np.float32)
    bqkv = np.asarray(bqkv, dtype=np.float32)
    Wproj = np.asarray(Wproj, dtype=np.float32)
    bproj = np.asarray(bproj, dtype=np.float32)
    assert x.shape == (B, T, C), x.shape

    nc = _get_nc()
    kk = np.arange(P)[:, None]
    qq = np.arange(512)[None, :]
    tril = (kk <= qq).astype(np.float32)

    in_maps = []
    for core in range(N_CORES):
        b, g = divmod(core, 2)
        sl = slice(g * GC, (g + 1) * GC)
        in_maps.append({
            "xT": np.ascontiguousarray(x[b].T),
            "wq": np.ascontiguousarray(Wqkv[:, 0 * C:1 * C][:, sl]),
            "wk": np.ascontiguousarray(Wqkv[:, 1 * C:2 * C][:, sl]),
            "wv": np.ascontiguousarray(Wqkv[:, 2 * C:3 * C][:, sl]),
            "bq": np.ascontiguousarray(bqkv[0 * C:1 * C][sl]),
            "bk": np.ascontiguousarray(bqkv[1 * C:2 * C][sl]),
            "bv": np.ascontiguousarray(bqkv[2 * C:3 * C][sl]),
            "wp": np.ascontiguousarray(Wproj[g * GC:(g + 1) * GC, :]),
            "pad": padding_mask[b].astype(np.float32),
            "mask": tril,
        })

    trace = bool(os.environ.get("BASS_KERNEL_TRACE"))
    res = run_bass_kernel_spmd(
        nc, in_maps, core_ids=list(range(N_CORES)), trace=trace)
    _cached["last_result"] = res

    out = np.empty((B, T, C), dtype=np.float32)
    for b in range(B):
        out[b] = res.results[2 * b]["out"] + res.results[2 * b + 1]["out"] + bproj
    return out



# revision 22
# speedup vs baseline: 1.5236x; 1.5236x over previous
"""Causal self-attention (B=4, T=2048, C=1024, H=16) on 8 Trainium2 NeuronCores.

Sharding: core = (batch b = core//2, head-group g = core%2, 8 heads each).

Key ideas over the dense formulation:
  - Padded keys (~50% of tokens) are compacted away on the host: K/V
    projections, S = K^T.T Q^T, exp and PV run only over the ~1024 valid
    keys per batch (padded to TC, a multiple of 128). Causality on the
    compacted index is still a contiguous prefix per query, so the k-tile
    loop bounds shrink; ragged tile edges are handled by host-precomputed
    bf16 {0,1} masks multiplied into P after the exp.
  - All matmul operands are bf16 (PSUM accumulation stays fp32): halves
    DMA/SBUF and makes LDWEIGHTS ~3x cheaper so it hides behind matmuls.
  - Single tile-pool scope, DMA issue order chosen so the PE starts ~3us
    in and has no >3.4us idle gap (keeps the HAM clock gate at 2.4GHz).
  - Softmax normalize: DVE reciprocal straight off the PSUM denominator
    row, then gpsimd broadcast + gpsimd multiplies (gpsimd is otherwise
    idle), freeing DVE for mask multiplies and PSUM evacuations.
  - Output projection of q-block qt-1 is interleaved into attention of
    block qt (fills PE bubbles left by the exp dependency chain).

Per-core program (compile-time specialized to the padding mask's compacted
tile structure; cached per structure):
  K^T proj -> V proj -> Q^T proj -> for qt: flash-style attention over the
  valid k-tiles with a LAG-3 S/exp -> PV software pipeline, 2 heads per
  PSUM tile; out-proj pieces trail by one q-block.
Host: per-batch token compaction, transposes, bf16 casts, ragged masks,
sums the two head-group partials per batch and adds bproj.
"""

import os
import sys

for _p in ("/opt/trn_rl_repo",):
    if _p not in sys.path:
        sys.path.append(_p)

import numpy as np
import ml_dtypes

B, T, C = 4, 2048, 1024
H, D = 16, 64
HPC = 8          # heads per core
GC = HPC * D     # 512 channels per core
N_CORES = 8
P = 128
NT = T // 512    # 4 q-blocks of 512
MT = GC // 128   # 4 m-tiles (head pairs)
CT = C // 128    # 8 contraction tiles

BF = ml_dtypes.bfloat16
_cached = {}


def _attn_meta(pm):
    """Compile-time step structure shared by all cores (union over batches).

    Returns (TC, steps, NBP, pad_tiles): steps[qt] = tuple of
    (ktc, q0, qe, mask_idx); columns [q0, qe) of the tile get the ragged
    causal mask multiplied in (mask_idx == -1: tile fully valid, no mask).
    Columns >= qe are fully causal-valid in every batch; pad rows beyond the
    valid count are killed by zeroing their Vp rows (incl. the ones column),
    so they never need masking. pad_tiles lists k-tiles with pad slots in
    some batch."""
    idx = [np.nonzero(pm[b])[0] for b in range(B)]
    cnt = [len(i) for i in idx]
    TCT = -(-max(cnt) // P)
    TC = TCT * P
    INF = 1 << 30
    first = np.full((B, TCT), INF, np.int64)
    last = np.full((B, TCT), -1, np.int64)
    haspad = np.zeros((B, TCT), bool)
    for b in range(B):
        for t in range(TCT):
            lo, hi = t * P, min((t + 1) * P, cnt[b])
            if lo < cnt[b]:
                first[b, t] = idx[b][lo]
                last[b, t] = idx[b][hi - 1]
            haspad[b, t] = (t + 1) * P > cnt[b]
    steps = []
    nmask = 0
    for qt in range(NT):
        row = []
        for ktc in range(TCT):
            fmin = int(first[:, ktc].min())
            if fmin >= (qt + 1) * 512:
                continue
            q0 = max(0, fmin - qt * 512) & ~7
            lmax = int(last[:, ktc].max())
            if lmax > qt * 512:
                qe = min(512, (lmax - qt * 512 + 1 + 7) & ~7)
                mi = nmask
                nmask += 1
            else:
                qe = q0
                mi = -1
            row.append((ktc, q0, qe, mi))
        steps.append(tuple(row))
    pad_tiles = tuple(t for t in range(TCT) if haspad[:, t].any())
    return TC, tuple(steps), nmask, pad_tiles


def _build(TC, steps, NBP, pad_tiles):
    import concourse.tile as tile
    from concourse import bacc, mybir
    import concourse.bass as bass

    TCT = TC // P
    f32 = mybir.dt.float32
    bf = mybir.dt.bfloat16
    AF = mybir.ActivationFunctionType
    ADD = mybir.AluOpType.add
    MUL = mybir.AluOpType.mult

    nc = bacc.Bacc("TRN2", target_bir_lowering=False, debug=False)

    xT_d = nc.dram_tensor("xT", [C, T], bf, kind="ExternalInput")
    xkv_d = nc.dram_tensor("xkv", [C, TC], bf, kind="ExternalInput")
    wq_d = nc.dram_tensor("wq", [C, GC], bf, kind="ExternalInput")
    wk_d = nc.dram_tensor("wk", [C, GC], bf, kind="ExternalInput")
    wv_d = nc.dram_tensor("wv", [C, GC], bf, kind="ExternalInput")
    bq_d = nc.dram_tensor("bq", [GC], f32, kind="ExternalInput")
    bk_d = nc.dram_tensor("bk", [GC], f32, kind="ExternalInput")
    bv_d = nc.dram_tensor("bv", [GC], f32, kind="ExternalInput")
    wp_d = nc.dram_tensor("wp", [GC, C], bf, kind="ExternalInput")
    msk_d = nc.dram_tensor("msk", [max(NBP, 1), P, 512], bf, kind="ExternalInput")
    val_d = nc.dram_tensor("val", [TC], f32, kind="ExternalInput")
    out_d = nc.dram_tensor("out", [T, C], f32, kind="ExternalOutput")

    # matmul PSUM output is capped at one bank = 512 fp32 per partition
    k_chunks = [(c0, min(c0 + 512, TC)) for c0 in range(0, TC, 512)]

    with tile.TileContext(nc) as tc:
        with tc.tile_pool(name="persist", bufs=1) as persist, \
             tc.tile_pool(name="ps", bufs=2, space="PSUM") as ps_pool, \
             tc.tile_pool(name="ppool", bufs=6) as ppool, \
             tc.tile_pool(name="ypool", bufs=2) as ypool, \
             tc.tile_pool(name="rpool", bufs=2) as rpool, \
             tc.tile_pool(name="bcpool", bufs=2) as bcpool, \
             tc.tile_pool(name="otpool", bufs=3) as otpool:
            QT = persist.tile([P, MT, T], bf, tag="QT")
            KT = persist.tile([P, MT, TC], bf, tag="KT")
            Vp = persist.tile([P, TCT, HPC, D + 1], bf, tag="Vp")
            MS = persist.tile([P, max(NBP, 1), 512], bf, tag="MS")
            xq_s = persist.tile([P, CT, T], bf, tag="xq")
            xkv_s = persist.tile([P, CT, TC], bf, tag="xkv")
            wq_s = persist.tile([P, CT, GC], bf, tag="wq")
            wk_s = persist.tile([P, CT, GC], bf, tag="wk")
            wv_s = persist.tile([P, CT, GC], bf, tag="wv")
            wp_s = persist.tile([P, MT, C], bf, tag="wp")
            bq_s = persist.tile([P, MT], f32, tag="bq")
            bk_s = persist.tile([P, MT], f32, tag="bk")
            bv_s = persist.tile([P, GC], f32, tag="bv")
            val_s = persist.tile([P, TCT], f32, tag="val")

            # --- DMAs, in the order compute consumes them ---------------
            nc.sync.dma_start(val_s[:], val_d.rearrange("(t p) -> p t", p=P))
            nc.sync.dma_start(bq_s[:], bq_d.rearrange("(m p) -> p m", p=P))
            nc.sync.dma_start(bk_s[:], bk_d.rearrange("(m p) -> p m", p=P))
            bv_ap = bass.AP(tensor=bv_d[:].tensor, offset=0, ap=[[0, P], [1, GC]])
            nc.sync.dma_start(bv_s[:], bv_ap)
            wkr = wk_d.rearrange("(c p) n -> p c n", p=P)
            wvr = wv_d.rearrange("(c p) n -> p c n", p=P)
            wqr = wq_d.rearrange("(c p) n -> p c n", p=P)
            xkvr = xkv_d.rearrange("(c p) t -> p c t", p=P)
            xqr = xT_d.rearrange("(c p) t -> p c t", p=P)
            # K-proj inputs first: wk m-tile 0, then xkv chunks + rest of wk
            nc.sync.dma_start(wk_s[:, :, 0:P], wkr[:, :, 0:P])
            nc.sync.dma_start(xkv_s[:, 0:2, :], xkvr[:, 0:2, :])
            nc.sync.dma_start(wk_s[:, :, P:GC], wkr[:, :, P:GC])
            for c2 in range(2, CT, 2):
                nc.sync.dma_start(xkv_s[:, c2:c2 + 2, :], xkvr[:, c2:c2 + 2, :])
            for c2 in range(0, CT, 2):
                nc.sync.dma_start(wv_s[:, c2:c2 + 2, :], wvr[:, c2:c2 + 2, :])
            for c2 in range(0, CT, 2):
                nc.sync.dma_start(xq_s[:, c2:c2 + 2, :], xqr[:, c2:c2 + 2, :])
            for c2 in range(0, CT, 2):
                nc.sync.dma_start(wq_s[:, c2:c2 + 2, :], wqr[:, c2:c2 + 2, :])
            if NBP:
                nc.sync.dma_start(MS[:], msk_d.rearrange("n p q -> p n q"))
            nc.sync.dma_start(wp_s[:], wp_d.rearrange("(m p) n -> p m n", p=P))

            # ones column of Vp (softmax denominator accumulator row)
            nc.vector.memset(Vp[:, :, :, D:D + 1], 1.0)

            # ---------------- K^T projection ----------------------------
            for m in range(MT):
                for (c0, c1) in k_chunks:
                    sl = slice(c0, c1)
                    pk = ps_pool.tile([P, 512], f32, tag="SS", name="pk")
                    for c in range(CT):
                        nc.tensor.matmul(
                            pk[:, 0:c1 - c0], wk_s[:, c, m * P:(m + 1) * P],
                            xkv_s[:, c, sl],
                            start=(c == 0), stop=(c == CT - 1))
                    nc.vector.tensor_scalar(
                        out=KT[:, m, sl], in0=pk[:, 0:c1 - c0],
                        scalar1=bk_s[:, m:m + 1], scalar2=None, op0=ADD)

            # ---------------- V projection ------------------------------
            for tt in range(TCT):
                pv = ps_pool.tile([P, GC], f32, tag="SS")
                for c in range(CT):
                    nc.tensor.matmul(
                        pv[:], xkv_s[:, c, tt * P:(tt + 1) * P], wv_s[:, c, :],
                        start=(c == 0), stop=(c == CT - 1))
                nc.vector.tensor_add(
                    Vp[:, tt, :, 0:D],
                    pv[:].rearrange("p (h d) -> p h d", h=HPC),
                    bv_s[:].rearrange("p (h d) -> p h d", h=HPC))
                if tt in pad_tiles:
                    # zero V rows + ones column of pad slots: they then
                    # contribute nothing to numerator or denominator
                    nc.vector.tensor_scalar(
                        out=Vp[:, tt, :, :], in0=Vp[:, tt, :, :],
                        scalar1=val_s[:, tt:tt + 1], scalar2=None, op0=MUL)

            # ---------------- Q^T projection (scaled by 1/8) ------------
            for m in range(MT):
                for nt in range(NT):
                    sl = slice(nt * 512, (nt + 1) * 512)
                    pq = ps_pool.tile([P, 512], f32, tag="SS", name="pq")
                    for c in range(CT):
                        nc.tensor.matmul(
                            pq[:], wq_s[:, c, m * P:(m + 1) * P], xq_s[:, c, sl],
                            start=(c == 0), stop=(c == CT - 1))
                    nc.vector.tensor_scalar(
                        out=QT[:, m, sl], in0=pq[:],
                        scalar1=bq_s[:, m:m + 1], scalar2=0.125,
                        op0=ADD, op1=MUL)

            # ---------------- attention + out-projection ----------------
            def proj_piece(qt_, yT_, ts):
                tt = qt_ * 4 + ts
                for nh in range(2):
                    pp_ = ps_pool.tile([P, 512], f32, tag="OO")
                    for cj in range(MT):
                        nc.tensor.matmul(
                            pp_[:], yT_[:, cj, ts * P:(ts + 1) * P],
                            wp_s[:, cj, nh * 512:(nh + 1) * 512],
                            start=(cj == 0), stop=(cj == MT - 1))
                    ot = otpool.tile([P, 512], f32, tag="ot")
                    if nh == 0:
                        nc.vector.tensor_copy(ot[:], pp_[:])
                    else:
                        nc.scalar.activation(ot[:], pp_[:], AF.Copy)
                    nc.sync.dma_start(
                        out_d[tt * P:(tt + 1) * P, nh * 512:(nh + 1) * 512], ot[:])

            yTq_prev = None
            for qt in range(NT):
                yTq = ypool.tile([P, MT, 512], bf, tag="yTq")
                srow = steps[qt]
                ns = len(srow)
                OO_map = {}
                pend = []
                LAG = 3

                def normalize_and_aux(j_, qt=qt, yTq=yTq, yTq_prev=yTq_prev):
                    OO_ = OO_map[j_]
                    lraw = rpool.tile([1, 2, 512], f32, tag="lraw")
                    nc.vector.tensor_copy(lraw[0:1, :, :], OO_[D:D + 1, :, :])
                    rec = rpool.tile([1, 2, 512], f32, tag="rec")
                    nc.vector.reciprocal_approx_fast(
                        rec[0:1, :, :], lraw[0:1, :, :])
                    bc = bcpool.tile([P, 2, 512], f32, tag="bc")
                    nc.gpsimd.partition_broadcast(bc[:], rec[0:1, :, :], channels=P)
                    nc.vector.tensor_mul(yTq[0:D, j_, :], OO_[0:D, 0, :], bc[0:D, 0, :])
                    nc.vector.tensor_mul(yTq[D:P, j_, :], OO_[0:D, 1, :], bc[D:P, 1, :])
                    if yTq_prev is not None:
                        proj_piece(qt - 1, yTq_prev, j_)

                def emit_pv(entry):
                    j_, si_, z_, PP_ = entry
                    OO_ = OO_map[j_]
                    ktc = srow[si_][0]
                    last_ = (si_ == ns - 1)
                    for e in range(2):
                        nc.tensor.matmul(
                            OO_[:, e, z_:512], Vp[:, ktc, 2 * j_ + e, :],
                            PP_[:, e, z_:512],
                            start=(si_ == 0), stop=last_)
                    if last_:
                        normalize_and_aux(j_)

                for j in range(MT):
                    OO_map[j] = ps_pool.tile(
                        [D + 1, 2, 512], f32, tag="OO", name="OO")
                    for si, (ktc, q0, qe, mi) in enumerate(srow):
                        SS = ps_pool.tile([P, 2, 512], f32, tag="SS")
                        nc.tensor.matmul(
                            SS[:, 0, q0:512], KT[0:D, j, ktc * P:(ktc + 1) * P],
                            QT[0:D, j, qt * 512 + q0:(qt + 1) * 512],
                            start=True, stop=True)
                        nc.tensor.matmul(
                            SS[:, 1, q0:512], KT[D:P, j, ktc * P:(ktc + 1) * P],
                            QT[D:P, j, qt * 512 + q0:(qt + 1) * 512],
                            start=True, stop=True)
                        PP = ppool.tile([P, 2, 512], bf, tag="PP")
                        nc.scalar.activation(
                            PP[:, :, q0:512], SS[:, :, q0:512], AF.Exp)
                        if mi >= 0:
                            tm = MS[:, mi, q0:qe]
                            mask_b = bass.AP(
                                tensor=tm.tensor, offset=tm.offset,
                                ap=[list(tm.ap[0]), [0, 2], list(tm.ap[1])])
                            nc.vector.tensor_mul(
                                PP[:, :, q0:qe], PP[:, :, q0:qe], mask_b)
                        pend.append((j, si, q0, PP))
                        if len(pend) > LAG:
                            emit_pv(pend.pop(0))
                while pend:
                    emit_pv(pend.pop(0))
                yTq_prev = yTq
            for ts in range(4):
                proj_piece(NT - 1, yTq_prev, ts)

    nc.compile()
    return nc


def _get_nc(TC, steps, NBP, pad_tiles):
    key = (TC, steps, pad_tiles)
    if key not in _cached:
        _cached[key] = _build(TC, steps, NBP, pad_tiles)
    return _cached[key]


def kernel(x, padding_mask, Wqkv, bqkv, Wproj, bproj):
    from concourse.bass_utils import run_bass_kernel_spmd

    x = np.asarray(x, dtype=np.float32)
    padding_mask = np.asarray(padding_mask)
    Wqkv = np.asarray(Wqkv, dtype=np.float32)
    bqkv = np.asarray(bqkv, dtype=np.float32)
    Wproj = np.asarray(Wproj, dtype=np.float32)
    bproj = np.asarray(bproj, dtype=np.float32)
    assert x.shape == (B, T, C), x.shape

    TC, steps, NBP, pad_tiles = _attn_meta(padding_mask)
    nc = _get_nc(TC, steps, NBP, pad_tiles)

    Wq16 = Wqkv[:, 0 * C:1 * C].astype(BF)
    Wk16 = Wqkv[:, 1 * C:2 * C].astype(BF)
    Wv16 = Wqkv[:, 2 * C:3 * C].astype(BF)
    Wp16 = Wproj.astype(BF)

    in_maps = []
    per_batch = {}
    for b in range(B):
        idx = np.nonzero(padding_mask[b])[0]
        cnt = len(idx)
        key_pos = np.full(TC, 1 << 20, np.int64)
        key_pos[:cnt] = idx
        valid = np.zeros(TC, np.float32)
        valid[:cnt] = 1.0
        xkv = np.zeros((TC, C), np.float32)
        xkv[:cnt] = x[b][idx]
        masks = np.zeros((max(NBP, 1), P, 512), BF)
        for qt in range(NT):
            qpos = qt * 512 + np.arange(512)[None, :]
            for (ktc, q0, qe, mi) in steps[qt]:
                if mi >= 0:
                    kp = key_pos[ktc * P:(ktc + 1) * P][:, None]
                    masks[mi] = (kp <= qpos).astype(BF)
        per_batch[b] = (
            np.ascontiguousarray(x[b].T.astype(BF)),
            np.ascontiguousarray(xkv.T.astype(BF)),
            masks,
            valid,
        )

    for core in range(N_CORES):
        b, g = divmod(core, 2)
        sl = slice(g * GC, (g + 1) * GC)
        xT16, xkvT16, masks, valid = per_batch[b]
        in_maps.append({
            "xT": xT16,
            "xkv": xkvT16,
            "val": valid,
            "wq": np.ascontiguousarray(Wq16[:, sl]),
            "wk": np.ascontiguousarray(Wk16[:, sl]),
            "wv": np.ascontiguousarray(Wv16[:, sl]),
            "bq": np.ascontiguousarray(bqkv[0 * C:1 * C][sl]),
            "bk": np.ascontiguousarray(bqkv[1 * C:2 * C][sl]),
            "bv": np.ascontiguousarray(bqkv[2 * C:3 * C][sl]),
            "wp": np.ascontiguousarray(Wp16[g * GC:(g + 1) * GC, :]),
            "msk": masks,
        })

    trace = bool(os.environ.get("BASS_KERNEL_TRACE"))
    res = run_bass_kernel_spmd(
        nc, in_maps, core_ids=list(range(N_CORES)), trace=trace)
    _cached["last_result"] = res

    out = np.empty((B, T, C), dtype=np.float32)
    for b in range(B):
        out[b] = res.results[2 * b]["out"] + res.results[2 * b + 1]["out"] + bproj
    return out
